# revision 26
# baseline (speedup 1.0000x reference)
"""Trainium2 Bass kernel for nn_ModelRQuery_5806795784426.

Strategy (data-parallel over bags, 8 cores x 64 bags):
  - node_weight (cosine-sim softmax) is computed with the exact same eager
    jax ops as the reference, so the Huffman merge schedule derived from it
    is bit-faithful to the reference's argmin decisions on this backend.
  - The Huffman weight evolution is replayed on host (pure IEEE f32 adds on
    identical bits -> identical schedule), producing per-bag merge pairs.
  - Per bag the merges are list-scheduled into pair-rounds (2 merges/round
    -> M=128 rows, full PE array) with children >= DIST=5 rounds earlier,
    so every round's scatter->gather->cast chain (~11us of DMA/semaphore
    latency) hides under ~3 rounds (~19us) of PE work.
  - The device runs only the FULL rounds (every bag has 2 merges): the
    chain-bound tail (the last ~9 merges/bag, <=1 merge/round) is replayed
    on the host in f32.  Every device round is 100% slot-utilized, there
    is no root special case, and accuracy improves (f32 tail).  The host
    tail consumes exactly the outputs of the last DIST device rounds,
    which are DMA'd straight to the output tensor (no readout gather).
  - tanh is applied at PRODUCTION: DRAM feats rows hold tanh'd bf16
    features (leaves host-pre-tanh'd).
  - Transposes are off the PE's critical path: the gather is
    dma_gather(transpose=True) -- it gathers the 2x128 child rows by index
    AND writes them transposed (feature-major) in one SWDGE instruction.
    h uses 8 PE transposes (~56ns issue each) + fine-grained DVE casts.
  - PE order: FC1_0, FC1_1, then [T(h_r) x8, FC2_r, FC1_{r+2}] per round,
    software-pipelining the tanh/transpose/gather latency of round r under
    FC1 of later rounds.  fp8 DoubleRow matmuls (K=256/pass).
"""

import numpy as np

NB = 64      # bags per core
NN = 64      # nodes (leaves) per bag
D = 1024
NCORES = 8
DIST = 5     # min round separation between child production and consumption

_PROG = {}


def _build_program(R, SL, zero_bias):
    """R = device rounds (all full, 2 merges/bag/round).  Rounds < R-DIST
    scatter to feats; rounds >= R-DIST write straight to the output."""
    key = (R, SL, zero_bias, "v8")
    if key in _PROG:
        return _PROG[key]
    import concourse.bass as bass
    import concourse.bacc as bacc
    import concourse.tile as tile

    mybir = bass.mybir
    f32 = mybir.dt.float32
    bf16 = mybir.dt.bfloat16
    f8 = mybir.dt.float8e4
    i16 = mybir.dt.int16
    TANH = mybir.ActivationFunctionType.Tanh
    ADD = mybir.AluOpType.add
    DR = mybir.MatmulPerfMode.DoubleRow

    nc = bacc.Bacc(None, target_bir_lowering=False, num_swdge_queues=2)
    # feats rows: bag*SL + slot, tanh'd bf16; slots 0..63 = leaves
    # (host-prefilled tanh(rep)), slot 64+2r+s = tanh(output) of round r
    # in-round slot s (only rounds < R-DIST are stored).
    feats_d = nc.dram_tensor("feats", [NB * SL, D], bf16, kind="ExternalInput")
    w1t_d = nc.dram_tensor("w1t", [2 * D, D], f8, kind="ExternalInput")
    w2t_d = nc.dram_tensor("w2t", [D, D], f8, kind="ExternalInput")
    b1b_d = nc.dram_tensor("b1b", [128, D], f32, kind="ExternalInput")
    b2b_d = nc.dram_tensor("b2b", [128, D], f32, kind="ExternalInput")
    # int16 gather indices: round q, op h -> gidx[:, 16q+8h : 16q+8h+8] in
    # the SWDGE wrapped-16 layout (position n -> [n%16, n//16], replicated
    # across partition groups).
    gidx_d = nc.dram_tensor("gidx", [128, 16 * R], i16, kind="ExternalInput")
    ident_d = nc.dram_tensor("ident", [128, 128], bf16, kind="ExternalInput")
    # tanh'd outputs of the last DIST rounds: out[lb, 2t+s] = round R-DIST+t
    out_d = nc.dram_tensor("out", [NB, 2 * DIST, D], bf16, kind="ExternalOutput")

    with tile.TileContext(nc) as tc:
        with tc.tile_pool(name="const", bufs=1) as cp, \
             tc.tile_pool(name="xb", bufs=5) as xbp, \
             tc.tile_pool(name="xq", bufs=3) as xqp, \
             tc.tile_pool(name="hp", bufs=2) as hp, \
             tc.tile_pool(name="fp", bufs=2) as fp, \
             tc.tile_pool(name="tpp", bufs=2, space="PSUM") as pt, \
             tc.tile_pool(name="mmf", bufs=1, space="PSUM") as pmf, \
             tc.tile_pool(name="mmh", bufs=2, space="PSUM") as pmh:

            feats3 = feats_d[:].rearrange("(b s) d -> b s d", s=SL)

            gixs = cp.tile([128, 16 * R], i16)
            nc.sync.dma_start(out=gixs[:], in_=gidx_d[:])
            ident = cp.tile([128, 128], bf16)
            nc.sync.dma_start(out=ident[:], in_=ident_d[:])

            w1t = cp.tile([128, 8, 2, D], f8)
            nc.sync.dma_start(out=w1t[:], in_=w1t_d[:].rearrange("(c two p) d -> p c two d", two=2, p=128))
            w2t = cp.tile([128, 4, 2, D], f8)
            nc.sync.dma_start(out=w2t[:], in_=w2t_d[:].rearrange("(c two p) d -> p c two d", two=2, p=128))
            if not zero_bias:
                b1b = cp.tile([128, D], f32)
                nc.sync.dma_start(out=b1b[:], in_=b1b_d[:])
                b2b = cp.tile([128, D], f32)
                nc.sync.dma_start(out=b2b[:], in_=b2b_d[:])

            def emit_xgather(q):
                # ONE fused gather+transpose of both 128-row operand sets:
                # xb[p, c, j] = feats[gidx_q[j], 128c+p], columns 0:128 =
                # op0 children, 128:256 = op1.  One DMA (not two) so each
                # round consumes half the DMA completion semaphores -- the
                # Tile sem-pool recycle waits sit at the head of the GPSIMD
                # FIFO and gate the preps, so sem-recycle distance directly
                # bounds how early gathers can run.
                xb = xbp.tile([128, 8, 256], bf16, tag="xb")
                nc.gpsimd.dma_gather(
                    out_ap=xb[:],
                    in_ap=feats_d[:],
                    idxs_ap=gixs[:, 16 * q:16 * q + 16],
                    num_idxs=256, num_idxs_reg=256, elem_size=D,
                    transpose=True, queue_num=q % 2)
                return xb

            def emit_xcast(xb):
                # bf16 gathered-transposed operands -> fp8 lhsT (DVE)
                xq = xqp.tile([128, 8, 256], f8, tag="xq")
                nc.vector.tensor_copy(out=xq[:, 0:4, :], in_=xb[:, 0:4, :])
                nc.vector.tensor_copy(out=xq[:, 4:8, :], in_=xb[:, 4:8, :])
                return xq

            def emit_fc1(xq):
                # h[(s,b), :] = x @ W1 (x already tanh'd; K=2048, fp8 DR).
                # lhsT for logical chunk pair (2c, 2c+1): op = c//4,
                # chunks-within-op = 2c%8, column half = op side.
                h0 = pmh.tile([128, 512], f32, tag="h0")
                h1 = pmh.tile([128, 512], f32, tag="h1")
                htt = hp.tile([128, D], bf16, tag="htt")
                hbt = None if zero_bias else hp.tile([128, D], f32, tag="hbt")
                for hn, ht in ((1, h1), (0, h0)):
                    for c in range(8):
                        cc = (2 * c) % 8
                        js = slice(128 * (c // 4), 128 * (c // 4) + 128)
                        nc.tensor.matmul(ht[:], xq[:, cc:cc + 2, js],
                                         w1t[:, c, :, 512 * hn:512 * (hn + 1)],
                                         start=(c == 0), stop=(c == 7), perf_mode=DR)
                    cs = slice(512 * hn, 512 * (hn + 1))
                    if zero_bias:
                        nc.scalar.activation(out=htt[:, cs], in_=ht[:], func=TANH)
                    else:
                        nc.vector.tensor_tensor(out=hbt[:, cs], in0=ht[:], in1=b1b[:, cs], op=ADD)
                        nc.scalar.activation(out=htt[:, cs], in_=hbt[:, cs], func=TANH)
                return htt

            def emit_hT(htt):
                # 8 PE transposes (128x128 bf16, ~56ns issue each) + 4
                # fine-grained DVE cast-copies in FC2's consumption order
                hT = hp.tile([128, 8, 128], f8, tag="hT")
                for q in (1, 0):
                    ps = pt.tile([128, 4, 128], bf16, tag="htp")
                    for j in range(4):
                        c = 4 * q + j
                        nc.tensor.transpose(out=ps[:, j, :], in_=htt[:, 128 * c:128 * (c + 1)],
                                            identity=ident[:])
                    nc.vector.tensor_copy(out=hT[:, 4 * q:4 * q + 2, :], in_=ps[:, 0:2, :])
                    nc.vector.tensor_copy(out=hT[:, 4 * q + 2:4 * q + 4, :], in_=ps[:, 2:4, :])
                return hT

            def emit_fc2(hT, r):
                f0 = pmf.tile([128, 512], f32, tag="f0")
                f1 = pmf.tile([128, 512], f32, tag="f1")
                ftb = fp.tile([128, D], bf16, tag="ftb")
                fbt = None if zero_bias else fp.tile([128, D], f32, tag="fbt")
                for fn, ft in ((0, f0), (1, f1)):
                    for ci, c in enumerate((2, 3, 0, 1)):
                        nc.tensor.matmul(ft[:], hT[:, 2 * c:2 * c + 2, :],
                                         w2t[:, c, :, 512 * fn:512 * (fn + 1)],
                                         start=(ci == 0), stop=(ci == 3), perf_mode=DR)
                    cs = slice(512 * fn, 512 * (fn + 1))
                    if zero_bias:
                        nc.scalar.activation(out=ftb[:, cs], in_=ft[:], func=TANH)
                    else:
                        nc.vector.tensor_tensor(out=fbt[:, cs], in0=ft[:], in1=b2b[:, cs], op=ADD)
                        nc.scalar.activation(out=ftb[:, cs], in_=fbt[:, cs], func=TANH)
                # ONE fused scatter (both slots): out AP iterates (s, b, d)
                # to match ftb's partition order s*64+b; single DMA halves
                # the per-round DMA-semaphore pressure (see emit_xgather)
                if r < R - DIST:
                    # consumed by later device rounds
                    nc.sync.dma_start(
                        out=feats3[:, 64 + 2 * r:64 + 2 * r + 2, :].rearrange("b s d -> s b d"),
                        in_=ftb[:])
                else:
                    # consumed only by the host tail -> straight to output
                    t = r - (R - DIST)
                    nc.sync.dma_start(
                        out=out_d[:, 2 * t:2 * t + 2, :].rearrange("b s d -> s b d"),
                        in_=ftb[:])

            # ---- software-pipelined main loop ----
            xb = {q: emit_xgather(q) for q in range(min(DIST, R))}
            xq = {0: emit_xcast(xb.pop(0))}
            if 1 < R:
                xq[1] = emit_xcast(xb.pop(1))
            htt = {0: emit_fc1(xq.pop(0))}
            if 1 < R:
                htt[1] = emit_fc1(xq.pop(1))
            for r in range(R):
                hT = emit_hT(htt.pop(r))
                if r + 2 < R:
                    xq[r + 2] = emit_xcast(xb.pop(r + 2))
                emit_fc2(hT, r)
                if r + 2 < R:
                    htt[r + 2] = emit_fc1(xq.pop(r + 2))
                if r + DIST < R:
                    xb[r + DIST] = emit_xgather(r + DIST)

    nc.compile()
    _PROG[key] = nc
    return nc


def _node_weight_like_reference(rep, n_per_bag):
    """Bit-faithful mirror of the reference's eager node_weight computation
    (reference runs on CPU jax; mirror that exactly)."""
    import jax
    import jax.numpy as jnp
    cpu = jax.local_devices(backend="cpu")[0]
    with jax.default_device(cpu):
        d = rep.shape[-1]
        bags = jnp.asarray(np.ascontiguousarray(rep, dtype=np.float32)).reshape(-1, n_per_bag, d)
        norms = jnp.linalg.norm(bags, axis=-1)
        gram = jnp.einsum('bnd,bmd->bnm', bags, bags)
        sims = gram / jnp.maximum(norms[:, :, None] * norms[:, None, :], 1e-8)
        node_distance = sims.sum(axis=1)
        node_weight = jax.nn.softmax(node_distance, axis=-1)
        return np.asarray(node_weight).astype(np.float32)


def _huffman_schedule(w):
    """Replay the reference scan's weight bookkeeping (exact f32) and emit
    per-bag merge operand nodes: leaves 0..63, merge t -> 64+t."""
    B, n = w.shape
    wref = w.copy()
    alive = np.ones((B, n), bool)
    prov = np.tile(np.arange(n, dtype=np.int64), (B, 1))
    ar = np.arange(B)
    gl = np.zeros((B, n - 1), np.int64)
    gr = np.zeros((B, n - 1), np.int64)
    INF = np.float32(np.inf)
    for t in range(n - 1):
        wm = np.where(alive, wref, INF)
        i1 = np.argmin(wm, axis=1)
        wm2 = wm.copy()
        wm2[ar, i1] = INF
        i2 = np.argmin(wm2, axis=1)
        gl[:, t] = prov[ar, i1]
        gr[:, t] = prov[ar, i2]
        wref[ar, i1] = wm[ar, i1] + wm[ar, i2]
        alive[ar, i2] = False
        prov[ar, i1] = n + t
    return gl, gr


def _pack_rounds(gl, gr, n=NN, dist=DIST):
    """List-schedule each bag's n-1 merges into pair-rounds (2 independent
    merges per round; children must be done <= r-dist; priority = longest
    path to root).  The root merge is pinned last.  Returns
    (rounds_of, slot_of, R_full)."""
    B, m = gl.shape
    rounds_of = np.zeros((B, m), np.int64)
    slot_of = np.zeros((B, m), np.int64)
    last_nonroot = 0
    root_child_max = 0
    for b in range(B):
        cl, cr = gl[b], gr[b]
        parents = np.full(m, -1, np.int64)
        ndep = np.zeros(m, np.int32)
        for j in range(m):
            for s in (cl[j], cr[j]):
                if s >= n:
                    ndep[j] += 1
                    parents[s - n] = j
        height = np.zeros(m, np.int64)
        for j in range(m - 1, -1, -1):
            p = parents[j]
            if p >= 0:
                height[j] = height[p] + 1
        done = np.full(m, 10**9, np.int64)
        remaining = ndep.copy()
        scheduled = 0
        r = 0
        while scheduled < m:
            ready = [j for j in range(m)
                     if remaining[j] == 0 and done[j] == 10**9
                     and all((s < n or done[s - n] <= r - dist) for s in (cl[j], cr[j]))]
            ready.sort(key=lambda j: (-height[j], j))
            for s_idx, j in enumerate(ready[:2]):
                rounds_of[b, j] = r
                slot_of[b, j] = s_idx
                done[j] = r
                scheduled += 1
                p = parents[j]
                if p >= 0:
                    remaining[p] -= 1
            r += 1
            assert r < 8 * m, "packer stuck"
        last_nonroot = max(last_nonroot, rounds_of[b, :m - 1].max())
        for s in (cl[m - 1], cr[m - 1]):
            if s >= n:
                root_child_max = max(root_child_max, int(rounds_of[b, s - n]))
    root_round = max(last_nonroot + 1, root_child_max + dist)
    rounds_of[:, m - 1] = root_round
    slot_of[:, m - 1] = 0
    R = root_round + 1
    for b in range(B):
        for j in range(m):
            r = rounds_of[b, j]
            for s in (gl[b, j], gr[b, j]):
                if s >= n:
                    assert rounds_of[b, s - n] <= r - dist, \
                        f"dist-{dist} violated: bag {b} merge {j}"
    return rounds_of, slot_of, R


def _wrap16(arr):
    """Pack a flat int array of gather positions into the SWDGE wrapped-16
    idx layout [128, n/16]: position n -> [n%16, n//16], replicated across
    the 8 partition groups."""
    ncols = len(arr) // 16
    block = arr.astype(np.int16).reshape(ncols, 16).T  # [16, ncols]
    return np.tile(block, (8, 1))  # [128, ncols]


def _prepare(rep, fc1_w, fc1_b, fc2_w, fc2_b, rel_emb, n_per_bag, **kw):
    n_per_bag = int(n_per_bag)
    assert n_per_bag == NN and rep.shape[-1] == D
    rep = np.ascontiguousarray(rep, dtype=np.float32)

    w = _node_weight_like_reference(rep, n_per_bag)
    gl, gr = _huffman_schedule(w)
    rounds_of, slot_of, R_full = _pack_rounds(gl, gr)
    B, m = gl.shape

    # device cut: keep only the rounds where EVERY bag has 2 merges
    percnt = np.zeros((B, R_full), np.int64)
    for b in range(B):
        for j in range(m):
            percnt[b, rounds_of[b, j]] += 1
    fullr = (percnt == 2).all(axis=0)
    R = int(np.argmin(fullr)) if not fullr.all() else R_full
    assert R > DIST

    SL = 64 + 2 * (R - DIST)
    zb = (not np.any(np.asarray(fc1_b))) and (not np.any(np.asarray(fc2_b)))
    merge_slot = 64 + 2 * rounds_of + slot_of          # (B, m); valid r < R-DIST

    # host tail: merges at rounds >= R.  Their device-side children must be
    # exactly the outputs of rounds R-DIST..R-1 (those are never consumed on
    # device and are DMA'd to the output tensor).
    host_merges = [[j for j in range(m) if rounds_of[b, j] >= R] for b in range(B)]
    for b in range(B):
        hs = set(host_merges[b])
        for j in host_merges[b]:
            for s in (gl[b, j], gr[b, j]):
                if s >= NN and (s - NN) not in hs:
                    assert rounds_of[b, s - NN] >= R - DIST, \
                        f"host child of bag {b} produced too early"
                else:
                    assert s >= NN, f"leaf child in host tail of bag {b}"

    nc = _build_program(R, SL, zb)

    import ml_dtypes
    f8 = ml_dtypes.float8_e4m3fn
    w1t = np.ascontiguousarray(np.asarray(fc1_w, np.float32).T).astype(f8)   # (2D, D)
    w2t = np.ascontiguousarray(np.asarray(fc2_w, np.float32).T).astype(f8)   # (D, D)
    b1b = np.ascontiguousarray(np.broadcast_to(np.asarray(fc1_b, np.float32), (128, D)))
    b2b = np.ascontiguousarray(np.broadcast_to(np.asarray(fc2_b, np.float32), (128, D)))
    ident = np.eye(128, dtype=ml_dtypes.bfloat16)

    def node_row(b, node):
        lb = b % NB
        return lb * SL + (node if node < NN else merge_slot[b, node - NN])

    in_maps = []
    for c in range(NCORES):
        b0 = c * NB
        gidx = np.zeros((128, 16 * R), np.int16)
        for q in range(R):
            arr = np.zeros(256, np.int64)
            for lb in range(NB):
                b = b0 + lb
                js = np.where(rounds_of[b] == q)[0]
                assert len(js) == 2
                for j in js:
                    s = slot_of[b, j]
                    arr[s * NB + lb] = node_row(b, int(gl[b, j]))
                    arr[128 + s * NB + lb] = node_row(b, int(gr[b, j]))
            gidx[:, 16 * q:16 * q + 16] = _wrap16(arr)

        feats = np.zeros((NB * SL, D), ml_dtypes.bfloat16)
        leaves = np.tanh(rep[b0 * NN:(b0 + NB) * NN].reshape(NB, NN, D)).astype(ml_dtypes.bfloat16)
        feats.reshape(NB, SL, D)[:, :NN, :] = leaves
        in_maps.append({
            "feats": feats,
            "w1t": w1t, "w2t": w2t,
            "b1b": b1b, "b2b": b2b, "gidx": gidx, "ident": ident,
        })

    tail = {
        "gl": gl, "gr": gr, "rounds_of": rounds_of, "slot_of": slot_of,
        "R": R, "host_merges": host_merges,
    }
    return nc, in_maps, tail


def _host_tail(res, tail, rep, fc1_w, fc1_b, fc2_w, fc2_b, rel_emb):
    """Replay the chain-bound tail merges in f32 and produce the output."""
    gl, gr = tail["gl"], tail["gr"]
    rounds_of, slot_of = tail["rounds_of"], tail["slot_of"]
    host_merges = tail["host_merges"]
    R = tail["R"]
    B, m = gl.shape
    w1 = np.asarray(fc1_w, np.float32)    # (D, 2D)
    w2 = np.asarray(fc2_w, np.float32)    # (D, D)
    b1 = np.asarray(fc1_b, np.float32)
    b2 = np.asarray(fc2_b, np.float32)
    rel = np.asarray(rel_emb, np.float32)

    # tanh'd features of the last-DIST-round device merges, per (bag, node)
    feat = {}
    for c in range(NCORES):
        fout = np.asarray(res.results[c]["out"]).astype(np.float32)  # (NB, 2*DIST, D)
        for lb in range(NB):
            b = c * NB + lb
            for j in range(m):
                r = rounds_of[b, j]
                if R - DIST <= r < R:
                    t = 2 * (r - (R - DIST)) + slot_of[b, j]
                    feat[(b, NN + j)] = fout[lb, t]

    groups = {}
    for b in range(B):
        for j in host_merges[b]:
            groups.setdefault(int(rounds_of[b, j]), []).append((b, j))
    root_feat = np.zeros((B, D), np.float32)
    for q in sorted(groups):
        items = groups[q]
        x = np.empty((len(items), 2 * D), np.float32)
        for i, (b, j) in enumerate(items):
            x[i, :D] = feat[(b, int(gl[b, j]))]
            x[i, D:] = feat[(b, int(gr[b, j]))]
        h = np.tanh(x @ w1.T + b1)
        f = h @ w2.T + b2                 # raw features of the new nodes
        for i, (b, j) in enumerate(items):
            if j == m - 1:
                root_feat[b] = f[i]
            else:
                feat[(b, NN + j)] = np.tanh(f[i])
    scores = root_feat @ rel.T
    out = 1.0 / (1.0 + np.exp(-scores, dtype=np.float64))
    return np.ascontiguousarray(out.astype(np.float32))


def kernel(rep, fc1_w, fc1_b, fc2_w, fc2_b, rel_emb, n_per_bag, **kw):
    nc, in_maps, tail = _prepare(rep, fc1_w, fc1_b, fc2_w, fc2_b, rel_emb, n_per_bag)
    from concourse import bass_utils
    res = bass_utils.run_bass_kernel_spmd(nc, in_maps, core_ids=list(range(NCORES)))
    return _host_tail(res, tail, rep, fc1_w, fc1_b, fc2_w, fc2_b, rel_emb)


# revision 32
# speedup vs baseline: 1.8892x; 1.8892x over previous
"""Trainium2 Bass kernel for nn_ModelRQuery_5806795784426.

Strategy (data-parallel over bags, 8 cores x 64 bags):
  - node_weight (cosine-sim softmax) is computed with the exact same eager
    jax ops as the reference, so the Huffman merge schedule derived from it
    is bit-faithful to the reference's argmin decisions on this backend.
  - The Huffman weight evolution is replayed on host (pure IEEE f32 adds on
    identical bits -> identical schedule), producing per-bag merge pairs.
  - Per bag the merges are list-scheduled into pair-rounds (2 merges/round
    -> M=128 rows, full PE array) with children >= DIST=5 rounds earlier,
    so every round's scatter->gather->cast chain (~11us of DMA/semaphore
    latency) hides under ~3 rounds (~19us) of PE work.
  - The device runs only the FULL rounds (every bag has 2 merges): the
    chain-bound tail (the last ~9 merges/bag, <=1 merge/round) is replayed
    on the host in f32.  Every device round is 100% slot-utilized, there
    is no root special case, and accuracy improves (f32 tail).  The host
    tail consumes exactly the outputs of the last DIST device rounds,
    which are DMA'd straight to the output tensor (no readout gather).
  - tanh is applied at PRODUCTION: DRAM feats rows hold tanh'd bf16
    features (leaves host-pre-tanh'd).
  - Transposes are off the PE's critical path: the gather is
    dma_gather(transpose=True) -- it gathers the 2x128 child rows by index
    AND writes them transposed (feature-major) in one SWDGE instruction.
    h uses 8 PE transposes (~56ns issue each) + fine-grained DVE casts.
  - PE order: FC1_0, FC1_1, then [T(h_r) x8, FC2_r, FC1_{r+2}] per round,
    software-pipelining the tanh/transpose/gather latency of round r under
    FC1 of later rounds.  fp8 DoubleRow matmuls (K=256/pass).
"""

import numpy as np

NB = 64      # bags per core
NN = 64      # nodes (leaves) per bag
D = 1024
NCORES = 8
DIST = 8     # min round separation between child production and consumption

_PROG = {}


def _build_program(R, SL, zero_bias):
    """R = device rounds (all full, 2 merges/bag/round).  Rounds < R-DIST
    scatter to feats; rounds >= R-DIST write straight to the output."""
    key = (R, SL, zero_bias, "v9")
    if key in _PROG:
        return _PROG[key]
    import concourse.bass as bass
    import concourse.bacc as bacc
    import concourse.tile as tile

    mybir = bass.mybir
    f32 = mybir.dt.float32
    bf16 = mybir.dt.bfloat16
    f8 = mybir.dt.float8e4
    i16 = mybir.dt.int16
    TANH = mybir.ActivationFunctionType.Tanh
    ADD = mybir.AluOpType.add
    DR = mybir.MatmulPerfMode.DoubleRow

    nc = bacc.Bacc(None, target_bir_lowering=False, num_swdge_queues=2)
    # feats rows: bag*SL + slot, tanh'd bf16; slots 0..63 = leaves
    # (host-prefilled tanh(rep)), slot 64+2r+s = tanh(output) of round r
    # in-round slot s (only rounds < R-DIST are stored).
    feats_d = nc.dram_tensor("feats", [NB * SL, D], bf16, kind="ExternalInput")
    w1t_d = nc.dram_tensor("w1t", [2 * D, D], f8, kind="ExternalInput")
    w2t_d = nc.dram_tensor("w2t", [D, D], f8, kind="ExternalInput")
    b1b_d = nc.dram_tensor("b1b", [128, D], f32, kind="ExternalInput")
    b2b_d = nc.dram_tensor("b2b", [128, D], f32, kind="ExternalInput")
    # int16 gather indices: round q, op h -> gidx[:, 16q+8h : 16q+8h+8] in
    # the SWDGE wrapped-16 layout (position n -> [n%16, n//16], replicated
    # across partition groups).
    gidx_d = nc.dram_tensor("gidx", [128, 16 * R], i16, kind="ExternalInput")
    ident_d = nc.dram_tensor("ident", [128, 128], bf16, kind="ExternalInput")
    # tanh'd outputs of the last DIST rounds: out[lb, 2t+s] = round R-DIST+t
    out_d = nc.dram_tensor("out", [NB, 2 * DIST, D], bf16, kind="ExternalOutput")

    with tile.TileContext(nc) as tc:
        with tc.tile_pool(name="const", bufs=1) as cp, \
             tc.tile_pool(name="xb", bufs=DIST) as xbp, \
             tc.tile_pool(name="xq", bufs=3) as xqp, \
             tc.tile_pool(name="hp", bufs=2) as hp, \
             tc.tile_pool(name="fp", bufs=2) as fp, \
             tc.tile_pool(name="tpp", bufs=2, space="PSUM") as pt, \
             tc.tile_pool(name="mmf", bufs=1, space="PSUM") as pmf, \
             tc.tile_pool(name="mmh", bufs=2, space="PSUM") as pmh:

            feats3 = feats_d[:].rearrange("(b s) d -> b s d", s=SL)

            gixs = cp.tile([128, 16 * R], i16)
            nc.sync.dma_start(out=gixs[:], in_=gidx_d[:])
            ident = cp.tile([128, 128], bf16)
            nc.sync.dma_start(out=ident[:], in_=ident_d[:])

            w1t = cp.tile([128, 8, 2, D], f8)
            nc.sync.dma_start(out=w1t[:], in_=w1t_d[:].rearrange("(c two p) d -> p c two d", two=2, p=128))
            w2t = cp.tile([128, 4, 2, D], f8)
            nc.sync.dma_start(out=w2t[:], in_=w2t_d[:].rearrange("(c two p) d -> p c two d", two=2, p=128))
            if not zero_bias:
                b1b = cp.tile([128, D], f32)
                nc.sync.dma_start(out=b1b[:], in_=b1b_d[:])
                b2b = cp.tile([128, D], f32)
                nc.sync.dma_start(out=b2b[:], in_=b2b_d[:])

            def emit_xgather(q):
                # gather+transpose the two 128-row operand sets of round q:
                # xb[p, 8h+c, j] = feats[gidx_q_h[j], 128c+p]
                xb = xbp.tile([128, 16, 128], bf16, tag="xb")
                for h in (0, 1):
                    nc.gpsimd.dma_gather(
                        out_ap=xb[:, 8 * h:8 * h + 8, :],
                        in_ap=feats_d[:],
                        idxs_ap=gixs[:, 16 * q + 8 * h:16 * q + 8 * h + 8],
                        num_idxs=128, num_idxs_reg=128, elem_size=D,
                        transpose=True, queue_num=h)
                return xb

            def emit_xcast(xb):
                # bf16 gathered-transposed operands -> fp8 lhsT (DVE)
                xq = xqp.tile([128, 16, 128], f8, tag="xq")
                nc.vector.tensor_copy(out=xq[:, 0:8, :], in_=xb[:, 0:8, :])
                nc.vector.tensor_copy(out=xq[:, 8:16, :], in_=xb[:, 8:16, :])
                return xq

            def emit_fc1(xq):
                # h[(s,b), :] = x @ W1 (x already tanh'd; K=2048, fp8 DR)
                h0 = pmh.tile([128, 512], f32, tag="h0")
                h1 = pmh.tile([128, 512], f32, tag="h1")
                htt = hp.tile([128, D], bf16, tag="htt")
                hbt = None if zero_bias else hp.tile([128, D], f32, tag="hbt")
                for hn, ht in ((1, h1), (0, h0)):
                    for c in range(8):
                        nc.tensor.matmul(ht[:], xq[:, 2 * c:2 * c + 2, :],
                                         w1t[:, c, :, 512 * hn:512 * (hn + 1)],
                                         start=(c == 0), stop=(c == 7), perf_mode=DR)
                    cs = slice(512 * hn, 512 * (hn + 1))
                    if zero_bias:
                        nc.scalar.activation(out=htt[:, cs], in_=ht[:], func=TANH)
                    else:
                        nc.vector.tensor_tensor(out=hbt[:, cs], in0=ht[:], in1=b1b[:, cs], op=ADD)
                        nc.scalar.activation(out=htt[:, cs], in_=hbt[:, cs], func=TANH)
                return htt

            def emit_hT(htt):
                # 8 PE transposes (128x128 bf16, ~56ns issue each) + 4
                # fine-grained DVE cast-copies in FC2's consumption order
                hT = hp.tile([128, 8, 128], f8, tag="hT")
                for q in (1, 0):
                    ps = pt.tile([128, 4, 128], bf16, tag="htp")
                    for j in range(4):
                        c = 4 * q + j
                        nc.tensor.transpose(out=ps[:, j, :], in_=htt[:, 128 * c:128 * (c + 1)],
                                            identity=ident[:])
                    nc.vector.tensor_copy(out=hT[:, 4 * q:4 * q + 2, :], in_=ps[:, 0:2, :])
                    nc.vector.tensor_copy(out=hT[:, 4 * q + 2:4 * q + 4, :], in_=ps[:, 2:4, :])
                return hT

            def emit_fc2(hT, r):
                f0 = pmf.tile([128, 512], f32, tag="f0")
                f1 = pmf.tile([128, 512], f32, tag="f1")
                ftb = fp.tile([128, D], bf16, tag="ftb")
                fbt = None if zero_bias else fp.tile([128, D], f32, tag="fbt")
                for fn, ft in ((0, f0), (1, f1)):
                    for ci, c in enumerate((2, 3, 0, 1)):
                        nc.tensor.matmul(ft[:], hT[:, 2 * c:2 * c + 2, :],
                                         w2t[:, c, :, 512 * fn:512 * (fn + 1)],
                                         start=(ci == 0), stop=(ci == 3), perf_mode=DR)
                    cs = slice(512 * fn, 512 * (fn + 1))
                    if zero_bias:
                        nc.scalar.activation(out=ftb[:, cs], in_=ft[:], func=TANH)
                    else:
                        nc.vector.tensor_tensor(out=fbt[:, cs], in0=ft[:], in1=b2b[:, cs], op=ADD)
                        nc.scalar.activation(out=ftb[:, cs], in_=fbt[:, cs], func=TANH)
                if r < R - DIST:
                    # consumed by later device rounds
                    for s in range(2):
                        nc.sync.dma_start(out=feats3[:, 64 + 2 * r + s, :],
                                          in_=ftb[64 * s:64 * (s + 1), :])
                else:
                    # consumed only by the host tail -> straight to output
                    t = r - (R - DIST)
                    for s in range(2):
                        nc.sync.dma_start(out=out_d[:, 2 * t + s, :],
                                          in_=ftb[64 * s:64 * (s + 1), :])

            # ---- software-pipelined main loop ----
            xb = {q: emit_xgather(q) for q in range(min(DIST, R))}
            xq = {0: emit_xcast(xb.pop(0))}
            if 1 < R:
                xq[1] = emit_xcast(xb.pop(1))
            htt = {0: emit_fc1(xq.pop(0))}
            if 1 < R:
                htt[1] = emit_fc1(xq.pop(1))
            for r in range(R):
                hT = emit_hT(htt.pop(r))
                if r + 2 < R:
                    xq[r + 2] = emit_xcast(xb.pop(r + 2))
                emit_fc2(hT, r)
                if r + 2 < R:
                    htt[r + 2] = emit_fc1(xq.pop(r + 2))
                if r + DIST < R:
                    xb[r + DIST] = emit_xgather(r + DIST)

    nc.compile()
    _PROG[key] = nc
    return nc


def _node_weight_like_reference(rep, n_per_bag):
    """Bit-faithful mirror of the reference's eager node_weight computation
    (reference runs on CPU jax; mirror that exactly)."""
    import jax
    import jax.numpy as jnp
    cpu = jax.local_devices(backend="cpu")[0]
    with jax.default_device(cpu):
        d = rep.shape[-1]
        bags = jnp.asarray(np.ascontiguousarray(rep, dtype=np.float32)).reshape(-1, n_per_bag, d)
        norms = jnp.linalg.norm(bags, axis=-1)
        gram = jnp.einsum('bnd,bmd->bnm', bags, bags)
        sims = gram / jnp.maximum(norms[:, :, None] * norms[:, None, :], 1e-8)
        node_distance = sims.sum(axis=1)
        node_weight = jax.nn.softmax(node_distance, axis=-1)
        return np.asarray(node_weight).astype(np.float32)


def _huffman_schedule(w):
    """Replay the reference scan's weight bookkeeping (exact f32) and emit
    per-bag merge operand nodes: leaves 0..63, merge t -> 64+t."""
    B, n = w.shape
    wref = w.copy()
    alive = np.ones((B, n), bool)
    prov = np.tile(np.arange(n, dtype=np.int64), (B, 1))
    ar = np.arange(B)
    gl = np.zeros((B, n - 1), np.int64)
    gr = np.zeros((B, n - 1), np.int64)
    INF = np.float32(np.inf)
    for t in range(n - 1):
        wm = np.where(alive, wref, INF)
        i1 = np.argmin(wm, axis=1)
        wm2 = wm.copy()
        wm2[ar, i1] = INF
        i2 = np.argmin(wm2, axis=1)
        gl[:, t] = prov[ar, i1]
        gr[:, t] = prov[ar, i2]
        wref[ar, i1] = wm[ar, i1] + wm[ar, i2]
        alive[ar, i2] = False
        prov[ar, i1] = n + t
    return gl, gr


def _pack_rounds(gl, gr, n=NN, dist=DIST):
    """List-schedule each bag's n-1 merges into pair-rounds (2 independent
    merges per round; children must be done <= r-dist; priority = longest
    path to root).  The root merge is pinned last.  Returns
    (rounds_of, slot_of, R_full)."""
    B, m = gl.shape
    rounds_of = np.zeros((B, m), np.int64)
    slot_of = np.zeros((B, m), np.int64)
    last_nonroot = 0
    root_child_max = 0
    for b in range(B):
        cl, cr = gl[b], gr[b]
        parents = np.full(m, -1, np.int64)
        ndep = np.zeros(m, np.int32)
        for j in range(m):
            for s in (cl[j], cr[j]):
                if s >= n:
                    ndep[j] += 1
                    parents[s - n] = j
        height = np.zeros(m, np.int64)
        for j in range(m - 1, -1, -1):
            p = parents[j]
            if p >= 0:
                height[j] = height[p] + 1
        done = np.full(m, 10**9, np.int64)
        remaining = ndep.copy()
        scheduled = 0
        r = 0
        while scheduled < m:
            ready = [j for j in range(m)
                     if remaining[j] == 0 and done[j] == 10**9
                     and all((s < n or done[s - n] <= r - dist) for s in (cl[j], cr[j]))]
            ready.sort(key=lambda j: (-height[j], j))
            for s_idx, j in enumerate(ready[:2]):
                rounds_of[b, j] = r
                slot_of[b, j] = s_idx
                done[j] = r
                scheduled += 1
                p = parents[j]
                if p >= 0:
                    remaining[p] -= 1
            r += 1
            assert r < 8 * m, "packer stuck"
        last_nonroot = max(last_nonroot, rounds_of[b, :m - 1].max())
        for s in (cl[m - 1], cr[m - 1]):
            if s >= n:
                root_child_max = max(root_child_max, int(rounds_of[b, s - n]))
    root_round = max(last_nonroot + 1, root_child_max + dist)
    rounds_of[:, m - 1] = root_round
    slot_of[:, m - 1] = 0
    R = root_round + 1
    for b in range(B):
        for j in range(m):
            r = rounds_of[b, j]
            for s in (gl[b, j], gr[b, j]):
                if s >= n:
                    assert rounds_of[b, s - n] <= r - dist, \
                        f"dist-{dist} violated: bag {b} merge {j}"
    return rounds_of, slot_of, R


def _wrap16(arr):
    """Pack a flat int array of gather positions into the SWDGE wrapped-16
    idx layout [128, n/16]: position n -> [n%16, n//16], replicated across
    the 8 partition groups."""
    ncols = len(arr) // 16
    block = arr.astype(np.int16).reshape(ncols, 16).T  # [16, ncols]
    return np.tile(block, (8, 1))  # [128, ncols]


def _prepare(rep, fc1_w, fc1_b, fc2_w, fc2_b, rel_emb, n_per_bag, **kw):
    n_per_bag = int(n_per_bag)
    assert n_per_bag == NN and rep.shape[-1] == D
    rep = np.ascontiguousarray(rep, dtype=np.float32)

    w = _node_weight_like_reference(rep, n_per_bag)
    gl, gr = _huffman_schedule(w)
    rounds_of, slot_of, R_full = _pack_rounds(gl, gr)
    B, m = gl.shape

    # device cut: keep only the rounds where EVERY bag has 2 merges
    percnt = np.zeros((B, R_full), np.int64)
    for b in range(B):
        for j in range(m):
            percnt[b, rounds_of[b, j]] += 1
    fullr = (percnt == 2).all(axis=0)
    R = int(np.argmin(fullr)) if not fullr.all() else R_full
    assert R > DIST

    SL = 64 + 2 * (R - DIST)
    zb = (not np.any(np.asarray(fc1_b))) and (not np.any(np.asarray(fc2_b)))
    merge_slot = 64 + 2 * rounds_of + slot_of          # (B, m); valid r < R-DIST

    # host tail: merges at rounds >= R.  Their device-side children must be
    # exactly the outputs of rounds R-DIST..R-1 (those are never consumed on
    # device and are DMA'd to the output tensor).
    host_merges = [[j for j in range(m) if rounds_of[b, j] >= R] for b in range(B)]
    for b in range(B):
        hs = set(host_merges[b])
        for j in host_merges[b]:
            for s in (gl[b, j], gr[b, j]):
                if s >= NN and (s - NN) not in hs:
                    assert rounds_of[b, s - NN] >= R - DIST, \
                        f"host child of bag {b} produced too early"
                else:
                    assert s >= NN, f"leaf child in host tail of bag {b}"

    nc = _build_program(R, SL, zb)

    import ml_dtypes
    f8 = ml_dtypes.float8_e4m3fn
    w1t = np.ascontiguousarray(np.asarray(fc1_w, np.float32).T).astype(f8)   # (2D, D)
    w2t = np.ascontiguousarray(np.asarray(fc2_w, np.float32).T).astype(f8)   # (D, D)
    b1b = np.ascontiguousarray(np.broadcast_to(np.asarray(fc1_b, np.float32), (128, D)))
    b2b = np.ascontiguousarray(np.broadcast_to(np.asarray(fc2_b, np.float32), (128, D)))
    ident = np.eye(128, dtype=ml_dtypes.bfloat16)

    def node_row(b, node):
        lb = b % NB
        return lb * SL + (node if node < NN else merge_slot[b, node - NN])

    in_maps = []
    for c in range(NCORES):
        b0 = c * NB
        gidx = np.zeros((128, 16 * R), np.int16)
        for q in range(R):
            for h in (0, 1):
                child = gl if h == 0 else gr
                arr = np.zeros(128, np.int64)
                for lb in range(NB):
                    b = b0 + lb
                    js = np.where(rounds_of[b] == q)[0]
                    assert len(js) == 2
                    for j in js:
                        s = slot_of[b, j]
                        arr[s * NB + lb] = node_row(b, int(child[b, j]))
                gidx[:, 16 * q + 8 * h:16 * q + 8 * h + 8] = _wrap16(arr)

        feats = np.zeros((NB * SL, D), ml_dtypes.bfloat16)
        leaves = np.tanh(rep[b0 * NN:(b0 + NB) * NN].reshape(NB, NN, D)).astype(ml_dtypes.bfloat16)
        feats.reshape(NB, SL, D)[:, :NN, :] = leaves
        in_maps.append({
            "feats": feats,
            "w1t": w1t, "w2t": w2t,
            "b1b": b1b, "b2b": b2b, "gidx": gidx, "ident": ident,
        })

    tail = {
        "gl": gl, "gr": gr, "rounds_of": rounds_of, "slot_of": slot_of,
        "R": R, "host_merges": host_merges,
    }
    return nc, in_maps, tail


def _host_tail(res, tail, rep, fc1_w, fc1_b, fc2_w, fc2_b, rel_emb):
    """Replay the chain-bound tail merges in f32 and produce the output."""
    gl, gr = tail["gl"], tail["gr"]
    rounds_of, slot_of = tail["rounds_of"], tail["slot_of"]
    host_merges = tail["host_merges"]
    R = tail["R"]
    B, m = gl.shape
    w1 = np.asarray(fc1_w, np.float32)    # (D, 2D)
    w2 = np.asarray(fc2_w, np.float32)    # (D, D)
    b1 = np.asarray(fc1_b, np.float32)
    b2 = np.asarray(fc2_b, np.float32)
    rel = np.asarray(rel_emb, np.float32)

    # tanh'd features of the last-DIST-round device merges, per (bag, node)
    feat = {}
    for c in range(NCORES):
        fout = np.asarray(res.results[c]["out"]).astype(np.float32)  # (NB, 2*DIST, D)
        for lb in range(NB):
            b = c * NB + lb
            for j in range(m):
                r = rounds_of[b, j]
                if R - DIST <= r < R:
                    t = 2 * (r - (R - DIST)) + slot_of[b, j]
                    feat[(b, NN + j)] = fout[lb, t]

    groups = {}
    for b in range(B):
        for j in host_merges[b]:
            groups.setdefault(int(rounds_of[b, j]), []).append((b, j))
    root_feat = np.zeros((B, D), np.float32)
    for q in sorted(groups):
        items = groups[q]
        x = np.empty((len(items), 2 * D), np.float32)
        for i, (b, j) in enumerate(items):
            x[i, :D] = feat[(b, int(gl[b, j]))]
            x[i, D:] = feat[(b, int(gr[b, j]))]
        h = np.tanh(x @ w1.T + b1)
        f = h @ w2.T + b2                 # raw features of the new nodes
        for i, (b, j) in enumerate(items):
            if j == m - 1:
                root_feat[b] = f[i]
            else:
                feat[(b, NN + j)] = np.tanh(f[i])
    scores = root_feat @ rel.T
    out = 1.0 / (1.0 + np.exp(-scores, dtype=np.float64))
    return np.ascontiguousarray(out.astype(np.float32))


def kernel(rep, fc1_w, fc1_b, fc2_w, fc2_b, rel_emb, n_per_bag, **kw):
    nc, in_maps, tail = _prepare(rep, fc1_w, fc1_b, fc2_w, fc2_b, rel_emb, n_per_bag)
    from concourse import bass_utils
    res = bass_utils.run_bass_kernel_spmd(nc, in_maps, core_ids=list(range(NCORES)))
    return _host_tail(res, tail, rep, fc1_w, fc1_b, fc2_w, fc2_b, rel_emb)


# revision 38
# speedup vs baseline: 1.9946x; 1.0558x over previous
"""Trainium2 Bass kernel for nn_ModelRQuery_5806795784426.

Strategy (data-parallel over bags, 8 cores x 64 bags):
  - node_weight (cosine-sim softmax) is computed with the exact same eager
    jax ops as the reference, so the Huffman merge schedule derived from it
    is bit-faithful to the reference's argmin decisions on this backend.
  - The Huffman weight evolution is replayed on host (pure IEEE f32 adds on
    identical bits -> identical schedule), producing per-bag merge pairs.
  - Per bag the merges are list-scheduled into pair-rounds (2 merges/round
    -> M=128 rows, full PE array) with children >= DIST=5 rounds earlier,
    so every round's scatter->gather->cast chain (~11us of DMA/semaphore
    latency) hides under ~3 rounds (~19us) of PE work.
  - The device runs only the FULL rounds (every bag has 2 merges): the
    chain-bound tail (the last ~9 merges/bag, <=1 merge/round) is replayed
    on the host in f32.  Every device round is 100% slot-utilized, there
    is no root special case, and accuracy improves (f32 tail).  The host
    tail consumes exactly the outputs of the last DIST device rounds,
    which are DMA'd straight to the output tensor (no readout gather).
  - tanh is applied at PRODUCTION: DRAM feats rows hold tanh'd bf16
    features (leaves host-pre-tanh'd).
  - Transposes are off the PE's critical path: the gather is
    dma_gather(transpose=True) -- it gathers the 2x128 child rows by index
    AND writes them transposed (feature-major) in one SWDGE instruction.
    h uses 8 PE transposes (~56ns issue each) + fine-grained DVE casts.
  - PE order: FC1_0, FC1_1, then [T(h_r) x8, FC2_r, FC1_{r+2}] per round,
    software-pipelining the tanh/transpose/gather latency of round r under
    FC1 of later rounds.  fp8 DoubleRow matmuls (K=256/pass).
"""

import numpy as np

NB = 64      # bags per core
NN = 64      # nodes (leaves) per bag
D = 1024
NCORES = 8
DIST = 8     # min round separation between child production and consumption

_PROG = {}


def _build_program(R, SL, zero_bias):
    """R = device rounds (all full, 2 merges/bag/round).  Rounds < R-DIST
    scatter to feats; rounds >= R-DIST write straight to the output."""
    key = (R, SL, zero_bias, "v10")
    if key in _PROG:
        return _PROG[key]
    import concourse.bass as bass
    import concourse.bacc as bacc
    import concourse.tile as tile

    mybir = bass.mybir
    f32 = mybir.dt.float32
    bf16 = mybir.dt.bfloat16
    f8 = mybir.dt.float8e4
    i16 = mybir.dt.int16
    TANH = mybir.ActivationFunctionType.Tanh
    ADD = mybir.AluOpType.add
    DR = mybir.MatmulPerfMode.DoubleRow

    nc = bacc.Bacc(None, target_bir_lowering=False, num_swdge_queues=2)
    # feats rows: bag*SL + slot, tanh'd bf16; slots 0..63 = leaves
    # (host-prefilled tanh(rep)), slot 64+2r+s = tanh(output) of round r
    # in-round slot s (only rounds < R-DIST are stored).
    feats_d = nc.dram_tensor("feats", [NB * SL, D], bf16, kind="ExternalInput")
    w1t_d = nc.dram_tensor("w1t", [2 * D, D], f8, kind="ExternalInput")
    w2t_d = nc.dram_tensor("w2t", [D, D], f8, kind="ExternalInput")
    b1b_d = nc.dram_tensor("b1b", [128, D], f32, kind="ExternalInput")
    b2b_d = nc.dram_tensor("b2b", [128, D], f32, kind="ExternalInput")
    # int16 gather indices: round q, op h -> gidx[:, 16q+8h : 16q+8h+8] in
    # the SWDGE wrapped-16 layout (position n -> [n%16, n//16], replicated
    # across partition groups).
    gidx_d = nc.dram_tensor("gidx", [128, 16 * R], i16, kind="ExternalInput")
    # rounds 0/1 operands, host-pre-gathered/transposed/cast (their children
    # are all leaves) -> FC1_0 starts ~20us earlier (no DGE chain at start)
    xq01_d = nc.dram_tensor("xq01", [128, 32 * 128], f8, kind="ExternalInput")
    ident_d = nc.dram_tensor("ident", [128, 128], bf16, kind="ExternalInput")
    # tanh'd outputs of the last DIST rounds: out[lb, 2t+s] = round R-DIST+t
    out_d = nc.dram_tensor("out", [NB, 2 * DIST, D], bf16, kind="ExternalOutput")

    with tile.TileContext(nc) as tc:
        with tc.tile_pool(name="const", bufs=1) as cp, \
             tc.tile_pool(name="xb", bufs=DIST) as xbp, \
             tc.tile_pool(name="xq", bufs=3) as xqp, \
             tc.tile_pool(name="hp", bufs=2) as hp, \
             tc.tile_pool(name="fp", bufs=2) as fp, \
             tc.tile_pool(name="tpp", bufs=2, space="PSUM") as pt, \
             tc.tile_pool(name="mmf", bufs=1, space="PSUM") as pmf, \
             tc.tile_pool(name="mmh", bufs=2, space="PSUM") as pmh:

            feats3 = feats_d[:].rearrange("(b s) d -> b s d", s=SL)

            gixs = cp.tile([128, 16 * R], i16)
            nc.sync.dma_start(out=gixs[:], in_=gidx_d[:])
            xq01 = cp.tile([128, 32, 128], f8)
            nc.sync.dma_start(out=xq01[:], in_=xq01_d[:].rearrange("p (c j) -> p c j", j=128))
            ident = cp.tile([128, 128], bf16)
            nc.sync.dma_start(out=ident[:], in_=ident_d[:])

            w1t = cp.tile([128, 8, 2, D], f8)
            nc.sync.dma_start(out=w1t[:], in_=w1t_d[:].rearrange("(c two p) d -> p c two d", two=2, p=128))
            w2t = cp.tile([128, 4, 2, D], f8)
            nc.sync.dma_start(out=w2t[:], in_=w2t_d[:].rearrange("(c two p) d -> p c two d", two=2, p=128))
            if not zero_bias:
                b1b = cp.tile([128, D], f32)
                nc.sync.dma_start(out=b1b[:], in_=b1b_d[:])
                b2b = cp.tile([128, D], f32)
                nc.sync.dma_start(out=b2b[:], in_=b2b_d[:])

            def emit_xgather(q):
                # gather+transpose the two 128-row operand sets of round q:
                # xb[p, 8h+c, j] = feats[gidx_q_h[j], 128c+p]
                xb = xbp.tile([128, 16, 128], bf16, tag="xb")
                for h in (0, 1):
                    nc.gpsimd.dma_gather(
                        out_ap=xb[:, 8 * h:8 * h + 8, :],
                        in_ap=feats_d[:],
                        idxs_ap=gixs[:, 16 * q + 8 * h:16 * q + 8 * h + 8],
                        num_idxs=128, num_idxs_reg=128, elem_size=D,
                        transpose=True, queue_num=h)
                return xb

            def emit_xcast(xb):
                # bf16 gathered-transposed operands -> fp8 lhsT (DVE)
                xq = xqp.tile([128, 16, 128], f8, tag="xq")
                nc.vector.tensor_copy(out=xq[:, 0:8, :], in_=xb[:, 0:8, :])
                nc.vector.tensor_copy(out=xq[:, 8:16, :], in_=xb[:, 8:16, :])
                return xq

            def emit_fc1(xq, base=0):
                # h[(s,b), :] = x @ W1 (x already tanh'd; K=2048, fp8 DR)
                h0 = pmh.tile([128, 512], f32, tag="h0")
                h1 = pmh.tile([128, 512], f32, tag="h1")
                htt = hp.tile([128, D], bf16, tag="htt")
                hbt = None if zero_bias else hp.tile([128, D], f32, tag="hbt")
                for hn, ht in ((1, h1), (0, h0)):
                    for c in range(8):
                        cc = base + 2 * c
                        nc.tensor.matmul(ht[:], xq[:, cc:cc + 2, :],
                                         w1t[:, c, :, 512 * hn:512 * (hn + 1)],
                                         start=(c == 0), stop=(c == 7), perf_mode=DR)
                    cs = slice(512 * hn, 512 * (hn + 1))
                    if zero_bias:
                        nc.scalar.activation(out=htt[:, cs], in_=ht[:], func=TANH)
                    else:
                        nc.vector.tensor_tensor(out=hbt[:, cs], in0=ht[:], in1=b1b[:, cs], op=ADD)
                        nc.scalar.activation(out=htt[:, cs], in_=hbt[:, cs], func=TANH)
                return htt

            def emit_hT(htt):
                # 8 PE transposes (128x128 bf16, ~56ns issue each) + 4
                # fine-grained DVE cast-copies in FC2's consumption order
                hT = hp.tile([128, 8, 128], f8, tag="hT")
                for q in (1, 0):
                    ps = pt.tile([128, 4, 128], bf16, tag="htp")
                    for j in range(4):
                        c = 4 * q + j
                        nc.tensor.transpose(out=ps[:, j, :], in_=htt[:, 128 * c:128 * (c + 1)],
                                            identity=ident[:])
                    nc.vector.tensor_copy(out=hT[:, 4 * q:4 * q + 2, :], in_=ps[:, 0:2, :])
                    nc.vector.tensor_copy(out=hT[:, 4 * q + 2:4 * q + 4, :], in_=ps[:, 2:4, :])
                return hT

            def emit_fc2(hT, r):
                f0 = pmf.tile([128, 512], f32, tag="f0")
                f1 = pmf.tile([128, 512], f32, tag="f1")
                ftb = fp.tile([128, D], bf16, tag="ftb")
                fbt = None if zero_bias else fp.tile([128, D], f32, tag="fbt")
                for fn, ft in ((0, f0), (1, f1)):
                    for ci, c in enumerate((2, 3, 0, 1)):
                        nc.tensor.matmul(ft[:], hT[:, 2 * c:2 * c + 2, :],
                                         w2t[:, c, :, 512 * fn:512 * (fn + 1)],
                                         start=(ci == 0), stop=(ci == 3), perf_mode=DR)
                    cs = slice(512 * fn, 512 * (fn + 1))
                    if zero_bias:
                        nc.scalar.activation(out=ftb[:, cs], in_=ft[:], func=TANH)
                    else:
                        nc.vector.tensor_tensor(out=fbt[:, cs], in0=ft[:], in1=b2b[:, cs], op=ADD)
                        nc.scalar.activation(out=ftb[:, cs], in_=fbt[:, cs], func=TANH)
                if r < R - DIST:
                    # consumed by later device rounds
                    for s in range(2):
                        nc.sync.dma_start(out=feats3[:, 64 + 2 * r + s, :],
                                          in_=ftb[64 * s:64 * (s + 1), :])
                else:
                    # consumed only by the host tail -> straight to output
                    t = r - (R - DIST)
                    for s in range(2):
                        nc.sync.dma_start(out=out_d[:, 2 * t + s, :],
                                          in_=ftb[64 * s:64 * (s + 1), :])

            # ---- software-pipelined main loop ----
            # Per-iteration engine FIFOs (order matters -- in-order queues):
            #   PE:     [T(h_r) x8, FC1_{r+2}, FC2_r]
            #   DVE:    [xcast_{r+2} x2, hT copies_r x4]
            #   Scalar: [tanh h_{r+2} x2, tanh f_r x2]
            #   Sync:   [scatter_r]
            #   GpSimd: [gathers_{r+DIST}]
            # so no FC matmul ever waits behind a late-gated queue head; the
            # scatter chain lands ~2us later but DIST=8 gives ~5 rounds of
            # slack there.
            assert R >= 2
            xb = {q: emit_xgather(q) for q in range(2, min(DIST, R))}
            htt = {0: emit_fc1(xq01, base=0), 1: emit_fc1(xq01, base=16)}
            xq = {}
            for r in range(R):
                if r + 2 < R:
                    xq[r + 2] = emit_xcast(xb.pop(r + 2))
                hT = emit_hT(htt.pop(r))
                if r + 2 < R:
                    htt[r + 2] = emit_fc1(xq.pop(r + 2))
                emit_fc2(hT, r)
                if r + DIST < R:
                    xb[r + DIST] = emit_xgather(r + DIST)

    nc.compile()
    _PROG[key] = nc
    return nc


def _node_weight_like_reference(rep, n_per_bag):
    """Bit-faithful mirror of the reference's eager node_weight computation
    (reference runs on CPU jax; mirror that exactly)."""
    import jax
    import jax.numpy as jnp
    cpu = jax.local_devices(backend="cpu")[0]
    with jax.default_device(cpu):
        d = rep.shape[-1]
        bags = jnp.asarray(np.ascontiguousarray(rep, dtype=np.float32)).reshape(-1, n_per_bag, d)
        norms = jnp.linalg.norm(bags, axis=-1)
        gram = jnp.einsum('bnd,bmd->bnm', bags, bags)
        sims = gram / jnp.maximum(norms[:, :, None] * norms[:, None, :], 1e-8)
        node_distance = sims.sum(axis=1)
        node_weight = jax.nn.softmax(node_distance, axis=-1)
        return np.asarray(node_weight).astype(np.float32)


def _huffman_schedule(w):
    """Replay the reference scan's weight bookkeeping (exact f32) and emit
    per-bag merge operand nodes: leaves 0..63, merge t -> 64+t."""
    B, n = w.shape
    wref = w.copy()
    alive = np.ones((B, n), bool)
    prov = np.tile(np.arange(n, dtype=np.int64), (B, 1))
    ar = np.arange(B)
    gl = np.zeros((B, n - 1), np.int64)
    gr = np.zeros((B, n - 1), np.int64)
    INF = np.float32(np.inf)
    for t in range(n - 1):
        wm = np.where(alive, wref, INF)
        i1 = np.argmin(wm, axis=1)
        wm2 = wm.copy()
        wm2[ar, i1] = INF
        i2 = np.argmin(wm2, axis=1)
        gl[:, t] = prov[ar, i1]
        gr[:, t] = prov[ar, i2]
        wref[ar, i1] = wm[ar, i1] + wm[ar, i2]
        alive[ar, i2] = False
        prov[ar, i1] = n + t
    return gl, gr


def _pack_rounds(gl, gr, n=NN, dist=DIST):
    """List-schedule each bag's n-1 merges into pair-rounds (2 independent
    merges per round; children must be done <= r-dist; priority = longest
    path to root).  The root merge is pinned last.  Returns
    (rounds_of, slot_of, R_full)."""
    B, m = gl.shape
    rounds_of = np.zeros((B, m), np.int64)
    slot_of = np.zeros((B, m), np.int64)
    last_nonroot = 0
    root_child_max = 0
    for b in range(B):
        cl, cr = gl[b], gr[b]
        parents = np.full(m, -1, np.int64)
        ndep = np.zeros(m, np.int32)
        for j in range(m):
            for s in (cl[j], cr[j]):
                if s >= n:
                    ndep[j] += 1
                    parents[s - n] = j
        height = np.zeros(m, np.int64)
        for j in range(m - 1, -1, -1):
            p = parents[j]
            if p >= 0:
                height[j] = height[p] + 1
        done = np.full(m, 10**9, np.int64)
        remaining = ndep.copy()
        scheduled = 0
        r = 0
        while scheduled < m:
            ready = [j for j in range(m)
                     if remaining[j] == 0 and done[j] == 10**9
                     and all((s < n or done[s - n] <= r - dist) for s in (cl[j], cr[j]))]
            ready.sort(key=lambda j: (-height[j], j))
            for s_idx, j in enumerate(ready[:2]):
                rounds_of[b, j] = r
                slot_of[b, j] = s_idx
                done[j] = r
                scheduled += 1
                p = parents[j]
                if p >= 0:
                    remaining[p] -= 1
            r += 1
            assert r < 8 * m, "packer stuck"
        last_nonroot = max(last_nonroot, rounds_of[b, :m - 1].max())
        for s in (cl[m - 1], cr[m - 1]):
            if s >= n:
                root_child_max = max(root_child_max, int(rounds_of[b, s - n]))
    root_round = max(last_nonroot + 1, root_child_max + dist)
    rounds_of[:, m - 1] = root_round
    slot_of[:, m - 1] = 0
    R = root_round + 1
    for b in range(B):
        for j in range(m):
            r = rounds_of[b, j]
            for s in (gl[b, j], gr[b, j]):
                if s >= n:
                    assert rounds_of[b, s - n] <= r - dist, \
                        f"dist-{dist} violated: bag {b} merge {j}"
    return rounds_of, slot_of, R


def _wrap16(arr):
    """Pack a flat int array of gather positions into the SWDGE wrapped-16
    idx layout [128, n/16]: position n -> [n%16, n//16], replicated across
    the 8 partition groups."""
    ncols = len(arr) // 16
    block = arr.astype(np.int16).reshape(ncols, 16).T  # [16, ncols]
    return np.tile(block, (8, 1))  # [128, ncols]


def _prepare(rep, fc1_w, fc1_b, fc2_w, fc2_b, rel_emb, n_per_bag, **kw):
    n_per_bag = int(n_per_bag)
    assert n_per_bag == NN and rep.shape[-1] == D
    rep = np.ascontiguousarray(rep, dtype=np.float32)

    w = _node_weight_like_reference(rep, n_per_bag)
    gl, gr = _huffman_schedule(w)
    rounds_of, slot_of, R_full = _pack_rounds(gl, gr)
    B, m = gl.shape

    # device cut: keep only the rounds where EVERY bag has 2 merges
    percnt = np.zeros((B, R_full), np.int64)
    for b in range(B):
        for j in range(m):
            percnt[b, rounds_of[b, j]] += 1
    fullr = (percnt == 2).all(axis=0)
    R = int(np.argmin(fullr)) if not fullr.all() else R_full
    assert R > DIST

    SL = 64 + 2 * (R - DIST)
    zb = (not np.any(np.asarray(fc1_b))) and (not np.any(np.asarray(fc2_b)))
    merge_slot = 64 + 2 * rounds_of + slot_of          # (B, m); valid r < R-DIST

    # host tail: merges at rounds >= R.  Their device-side children must be
    # exactly the outputs of rounds R-DIST..R-1 (those are never consumed on
    # device and are DMA'd to the output tensor).
    host_merges = [[j for j in range(m) if rounds_of[b, j] >= R] for b in range(B)]
    for b in range(B):
        hs = set(host_merges[b])
        for j in host_merges[b]:
            for s in (gl[b, j], gr[b, j]):
                if s >= NN and (s - NN) not in hs:
                    assert rounds_of[b, s - NN] >= R - DIST, \
                        f"host child of bag {b} produced too early"
                else:
                    assert s >= NN, f"leaf child in host tail of bag {b}"

    nc = _build_program(R, SL, zb)

    import ml_dtypes
    f8 = ml_dtypes.float8_e4m3fn
    w1t = np.ascontiguousarray(np.asarray(fc1_w, np.float32).T).astype(f8)   # (2D, D)
    w2t = np.ascontiguousarray(np.asarray(fc2_w, np.float32).T).astype(f8)   # (D, D)
    b1b = np.ascontiguousarray(np.broadcast_to(np.asarray(fc1_b, np.float32), (128, D)))
    b2b = np.ascontiguousarray(np.broadcast_to(np.asarray(fc2_b, np.float32), (128, D)))
    ident = np.eye(128, dtype=ml_dtypes.bfloat16)

    def node_row(b, node):
        lb = b % NB
        return lb * SL + (node if node < NN else merge_slot[b, node - NN])

    in_maps = []
    for c in range(NCORES):
        b0 = c * NB
        gidx = np.zeros((128, 16 * R), np.int16)
        for q in range(R):
            for h in (0, 1):
                child = gl if h == 0 else gr
                arr = np.zeros(128, np.int64)
                for lb in range(NB):
                    b = b0 + lb
                    js = np.where(rounds_of[b] == q)[0]
                    assert len(js) == 2
                    for j in js:
                        s = slot_of[b, j]
                        arr[s * NB + lb] = node_row(b, int(child[b, j]))
                gidx[:, 16 * q + 8 * h:16 * q + 8 * h + 8] = _wrap16(arr)

        feats = np.zeros((NB * SL, D), ml_dtypes.bfloat16)
        leaves = np.tanh(rep[b0 * NN:(b0 + NB) * NN].reshape(NB, NN, D)).astype(ml_dtypes.bfloat16)
        feats.reshape(NB, SL, D)[:, :NN, :] = leaves

        # host-pre-gathered/transposed/cast operands for rounds 0/1 (all
        # children are leaves when DIST >= 2): xq01[p, 16q+8h+c, j] =
        # feats[child_row(q,h,j), 128c+p], bf16 -> fp8 like the device cast
        x01 = np.zeros((128, 32, 128), ml_dtypes.float8_e4m3fn)
        for q in (0, 1):
            for h in (0, 1):
                child = gl if h == 0 else gr
                arr = np.zeros(128, np.int64)
                for lb in range(NB):
                    b = b0 + lb
                    js = np.where(rounds_of[b] == q)[0]
                    for j in js:
                        ch = int(child[b, j])
                        assert ch < NN, "round 0/1 child must be a leaf"
                        arr[slot_of[b, j] * NB + lb] = lb * SL + ch
                rows = feats[arr].astype(np.float32)            # (128, 1024)
                xt = rows.T.reshape(8, 128, 128)                # (c, p, j)
                for c in range(8):
                    x01[:, 16 * q + 8 * h + c, :] = xt[c].astype(ml_dtypes.bfloat16).astype(ml_dtypes.float8_e4m3fn)

        in_maps.append({
            "feats": feats,
            "w1t": w1t, "w2t": w2t, "xq01": np.ascontiguousarray(x01.reshape(128, 32 * 128)),
            "b1b": b1b, "b2b": b2b, "gidx": gidx, "ident": ident,
        })

    tail = {
        "gl": gl, "gr": gr, "rounds_of": rounds_of, "slot_of": slot_of,
        "R": R, "host_merges": host_merges,
    }
    return nc, in_maps, tail


def _host_tail(res, tail, rep, fc1_w, fc1_b, fc2_w, fc2_b, rel_emb):
    """Replay the chain-bound tail merges in f32 and produce the output."""
    gl, gr = tail["gl"], tail["gr"]
    rounds_of, slot_of = tail["rounds_of"], tail["slot_of"]
    host_merges = tail["host_merges"]
    R = tail["R"]
    B, m = gl.shape
    w1 = np.asarray(fc1_w, np.float32)    # (D, 2D)
    w2 = np.asarray(fc2_w, np.float32)    # (D, D)
    b1 = np.asarray(fc1_b, np.float32)
    b2 = np.asarray(fc2_b, np.float32)
    rel = np.asarray(rel_emb, np.float32)

    # tanh'd features of the last-DIST-round device merges, per (bag, node)
    feat = {}
    for c in range(NCORES):
        fout = np.asarray(res.results[c]["out"]).astype(np.float32)  # (NB, 2*DIST, D)
        for lb in range(NB):
            b = c * NB + lb
            for j in range(m):
                r = rounds_of[b, j]
                if R - DIST <= r < R:
                    t = 2 * (r - (R - DIST)) + slot_of[b, j]
                    feat[(b, NN + j)] = fout[lb, t]

    groups = {}
    for b in range(B):
        for j in host_merges[b]:
            groups.setdefault(int(rounds_of[b, j]), []).append((b, j))
    root_feat = np.zeros((B, D), np.float32)
    for q in sorted(groups):
        items = groups[q]
        x = np.empty((len(items), 2 * D), np.float32)
        for i, (b, j) in enumerate(items):
            x[i, :D] = feat[(b, int(gl[b, j]))]
            x[i, D:] = feat[(b, int(gr[b, j]))]
        h = np.tanh(x @ w1.T + b1)
        f = h @ w2.T + b2                 # raw features of the new nodes
        for i, (b, j) in enumerate(items):
            if j == m - 1:
                root_feat[b] = f[i]
            else:
                feat[(b, NN + j)] = np.tanh(f[i])
    scores = root_feat @ rel.T
    out = 1.0 / (1.0 + np.exp(-scores, dtype=np.float64))
    return np.ascontiguousarray(out.astype(np.float32))


def kernel(rep, fc1_w, fc1_b, fc2_w, fc2_b, rel_emb, n_per_bag, **kw):
    nc, in_maps, tail = _prepare(rep, fc1_w, fc1_b, fc2_w, fc2_b, rel_emb, n_per_bag)
    from concourse import bass_utils
    res = bass_utils.run_bass_kernel_spmd(nc, in_maps, core_ids=list(range(NCORES)))
    return _host_tail(res, tail, rep, fc1_w, fc1_b, fc2_w, fc2_b, rel_emb)


# revision 40
# speedup vs baseline: 2.0038x; 1.0046x over previous
"""Trainium2 Bass kernel for nn_ModelRQuery_5806795784426.

Strategy (data-parallel over bags, 8 cores x 64 bags):
  - node_weight (cosine-sim softmax) is computed with the exact same eager
    jax ops as the reference, so the Huffman merge schedule derived from it
    is bit-faithful to the reference's argmin decisions on this backend.
  - The Huffman weight evolution is replayed on host (pure IEEE f32 adds on
    identical bits -> identical schedule), producing per-bag merge pairs.
  - Per bag the merges are list-scheduled into pair-rounds (2 merges/round
    -> M=128 rows, full PE array) with children >= DIST=5 rounds earlier,
    so every round's scatter->gather->cast chain (~11us of DMA/semaphore
    latency) hides under ~3 rounds (~19us) of PE work.
  - The device runs only the FULL rounds (every bag has 2 merges): the
    chain-bound tail (the last ~9 merges/bag, <=1 merge/round) is replayed
    on the host in f32.  Every device round is 100% slot-utilized, there
    is no root special case, and accuracy improves (f32 tail).  The host
    tail consumes exactly the outputs of the last DIST device rounds,
    which are DMA'd straight to the output tensor (no readout gather).
  - tanh is applied at PRODUCTION: DRAM feats rows hold tanh'd bf16
    features (leaves host-pre-tanh'd).
  - Transposes are off the PE's critical path: the gather is
    dma_gather(transpose=True) -- it gathers the 2x128 child rows by index
    AND writes them transposed (feature-major) in one SWDGE instruction.
    h uses 8 PE transposes (~56ns issue each) + fine-grained DVE casts.
  - PE order: FC1_0, FC1_1, then [T(h_r) x8, FC2_r, FC1_{r+2}] per round,
    software-pipelining the tanh/transpose/gather latency of round r under
    FC1 of later rounds.  fp8 DoubleRow matmuls (K=256/pass).
"""

import numpy as np

NB = 64      # bags per core
NN = 64      # nodes (leaves) per bag
D = 1024
NCORES = 8
DIST = 8     # min round separation between child production and consumption

_PROG = {}


def _build_program(R, SL, zero_bias):
    """R = device rounds (all full, 2 merges/bag/round).  Rounds < R-DIST
    scatter to feats; rounds >= R-DIST write straight to the output."""
    key = (R, SL, zero_bias, "v11")
    if key in _PROG:
        return _PROG[key]
    import concourse.bass as bass
    import concourse.bacc as bacc
    import concourse.tile as tile

    mybir = bass.mybir
    f32 = mybir.dt.float32
    bf16 = mybir.dt.bfloat16
    f8 = mybir.dt.float8e4
    i16 = mybir.dt.int16
    TANH = mybir.ActivationFunctionType.Tanh
    ADD = mybir.AluOpType.add
    DR = mybir.MatmulPerfMode.DoubleRow

    nc = bacc.Bacc(None, target_bir_lowering=False, num_swdge_queues=2)
    # feats rows: bag*SL + slot, tanh'd bf16; slots 0..63 = leaves
    # (host-prefilled tanh(rep)), slot 64+2r+s = tanh(output) of round r
    # in-round slot s (only rounds < R-DIST are stored).
    feats_d = nc.dram_tensor("feats", [NB * SL, D], bf16, kind="ExternalInput")
    w1t_d = nc.dram_tensor("w1t", [2 * D, D], f8, kind="ExternalInput")
    w2t_d = nc.dram_tensor("w2t", [D, D], f8, kind="ExternalInput")
    b1b_d = nc.dram_tensor("b1b", [128, D], f32, kind="ExternalInput")
    b2b_d = nc.dram_tensor("b2b", [128, D], f32, kind="ExternalInput")
    # int16 gather indices: round q, op h -> gidx[:, 16q+8h : 16q+8h+8] in
    # the SWDGE wrapped-16 layout (position n -> [n%16, n//16], replicated
    # across partition groups).
    gidx_d = nc.dram_tensor("gidx", [128, 16 * R], i16, kind="ExternalInput")
    # rounds 0/1 operands, host-pre-gathered/transposed/cast (their children
    # are all leaves) -> FC1_0 starts ~20us earlier (no DGE chain at start)
    xq01_d = nc.dram_tensor("xq01", [128, 32 * 128], f8, kind="ExternalInput")
    ident_d = nc.dram_tensor("ident", [128, 128], bf16, kind="ExternalInput")
    # tanh'd outputs of the last DIST rounds: out[lb, 2t+s] = round R-DIST+t
    out_d = nc.dram_tensor("out", [NB, 2 * DIST, D], bf16, kind="ExternalOutput")

    with tile.TileContext(nc) as tc:
        with tc.tile_pool(name="const", bufs=1) as cp, \
             tc.tile_pool(name="xb", bufs=DIST) as xbp, \
             tc.tile_pool(name="xq", bufs=3) as xqp, \
             tc.tile_pool(name="hp", bufs=2) as hp, \
             tc.tile_pool(name="fp", bufs=2) as fp, \
             tc.tile_pool(name="tpp", bufs=2, space="PSUM") as pt, \
             tc.tile_pool(name="mmf", bufs=2, space="PSUM") as pmf, \
             tc.tile_pool(name="mmh", bufs=1, space="PSUM") as pmh:

            feats3 = feats_d[:].rearrange("(b s) d -> b s d", s=SL)

            gixs = cp.tile([128, 16 * R], i16)
            nc.sync.dma_start(out=gixs[:], in_=gidx_d[:])
            xq01 = cp.tile([128, 32, 128], f8)
            nc.sync.dma_start(out=xq01[:], in_=xq01_d[:].rearrange("p (c j) -> p c j", j=128))
            ident = cp.tile([128, 128], bf16)
            nc.sync.dma_start(out=ident[:], in_=ident_d[:])

            w1t = cp.tile([128, 8, 2, D], f8)
            nc.sync.dma_start(out=w1t[:], in_=w1t_d[:].rearrange("(c two p) d -> p c two d", two=2, p=128))
            w2t = cp.tile([128, 4, 2, D], f8)
            nc.sync.dma_start(out=w2t[:], in_=w2t_d[:].rearrange("(c two p) d -> p c two d", two=2, p=128))
            if not zero_bias:
                b1b = cp.tile([128, D], f32)
                nc.sync.dma_start(out=b1b[:], in_=b1b_d[:])
                b2b = cp.tile([128, D], f32)
                nc.sync.dma_start(out=b2b[:], in_=b2b_d[:])

            def emit_xgather(q):
                # gather+transpose the two 128-row operand sets of round q:
                # xb[p, 8h+c, j] = feats[gidx_q_h[j], 128c+p]
                xb = xbp.tile([128, 16, 128], bf16, tag="xb")
                for h in (0, 1):
                    nc.gpsimd.dma_gather(
                        out_ap=xb[:, 8 * h:8 * h + 8, :],
                        in_ap=feats_d[:],
                        idxs_ap=gixs[:, 16 * q + 8 * h:16 * q + 8 * h + 8],
                        num_idxs=128, num_idxs_reg=128, elem_size=D,
                        transpose=True, queue_num=h)
                return xb

            def emit_xcast(xb):
                # bf16 gathered-transposed operands -> fp8 lhsT (DVE)
                xq = xqp.tile([128, 16, 128], f8, tag="xq")
                nc.vector.tensor_copy(out=xq[:, 0:8, :], in_=xb[:, 0:8, :])
                nc.vector.tensor_copy(out=xq[:, 8:16, :], in_=xb[:, 8:16, :])
                return xq

            def emit_fc1(xq, base=0):
                # h[(s,b), :] = x @ W1 (x already tanh'd; K=2048, fp8 DR)
                h0 = pmh.tile([128, 512], f32, tag="h0")
                h1 = pmh.tile([128, 512], f32, tag="h1")
                htt = hp.tile([128, D], bf16, tag="htt")
                hbt = None if zero_bias else hp.tile([128, D], f32, tag="hbt")
                for hn, ht in ((1, h1), (0, h0)):
                    for c in range(8):
                        cc = base + 2 * c
                        nc.tensor.matmul(ht[:], xq[:, cc:cc + 2, :],
                                         w1t[:, c, :, 512 * hn:512 * (hn + 1)],
                                         start=(c == 0), stop=(c == 7), perf_mode=DR)
                    cs = slice(512 * hn, 512 * (hn + 1))
                    if zero_bias:
                        nc.scalar.activation(out=htt[:, cs], in_=ht[:], func=TANH)
                    else:
                        nc.vector.tensor_tensor(out=hbt[:, cs], in0=ht[:], in1=b1b[:, cs], op=ADD)
                        nc.scalar.activation(out=htt[:, cs], in_=hbt[:, cs], func=TANH)
                return htt

            def emit_hT(htt):
                # 8 PE transposes (128x128 bf16, ~56ns issue each) + 4
                # fine-grained DVE cast-copies in FC2's consumption order
                hT = hp.tile([128, 8, 128], f8, tag="hT")
                for q in (1, 0):
                    ps = pt.tile([128, 4, 128], bf16, tag="htp")
                    for j in range(4):
                        c = 4 * q + j
                        nc.tensor.transpose(out=ps[:, j, :], in_=htt[:, 128 * c:128 * (c + 1)],
                                            identity=ident[:])
                    nc.vector.tensor_copy(out=hT[:, 4 * q:4 * q + 2, :], in_=ps[:, 0:2, :])
                    nc.vector.tensor_copy(out=hT[:, 4 * q + 2:4 * q + 4, :], in_=ps[:, 2:4, :])
                return hT

            def emit_fc2(hT, r):
                f0 = pmf.tile([128, 512], f32, tag="f0")
                f1 = pmf.tile([128, 512], f32, tag="f1")
                ftb = fp.tile([128, D], bf16, tag="ftb")
                fbt = None if zero_bias else fp.tile([128, D], f32, tag="fbt")
                for fn, ft in ((0, f0), (1, f1)):
                    for ci, c in enumerate((2, 3, 0, 1)):
                        nc.tensor.matmul(ft[:], hT[:, 2 * c:2 * c + 2, :],
                                         w2t[:, c, :, 512 * fn:512 * (fn + 1)],
                                         start=(ci == 0), stop=(ci == 3), perf_mode=DR)
                    cs = slice(512 * fn, 512 * (fn + 1))
                    if zero_bias:
                        nc.scalar.activation(out=ftb[:, cs], in_=ft[:], func=TANH)
                    else:
                        nc.vector.tensor_tensor(out=fbt[:, cs], in0=ft[:], in1=b2b[:, cs], op=ADD)
                        nc.scalar.activation(out=ftb[:, cs], in_=fbt[:, cs], func=TANH)
                if r < R - DIST:
                    # consumed by later device rounds
                    for s in range(2):
                        nc.sync.dma_start(out=feats3[:, 64 + 2 * r + s, :],
                                          in_=ftb[64 * s:64 * (s + 1), :])
                else:
                    # consumed only by the host tail -> straight to output
                    t = r - (R - DIST)
                    for s in range(2):
                        nc.sync.dma_start(out=out_d[:, 2 * t + s, :],
                                          in_=ftb[64 * s:64 * (s + 1), :])

            # ---- software-pipelined main loop ----
            # Per-iteration engine FIFOs (order matters -- in-order queues):
            #   PE:     [T(h_r) x8, FC1_{r+2}, FC2_r]
            #   DVE:    [xcast_{r+2} x2, hT copies_r x4]
            #   Scalar: [tanh h_{r+2} x2, tanh f_r x2]
            #   Sync:   [scatter_r]
            #   GpSimd: [gathers_{r+DIST}]
            # so no FC matmul ever waits behind a late-gated queue head; the
            # scatter chain lands ~2us later but DIST=8 gives ~5 rounds of
            # slack there.
            assert R >= 2
            xb = {q: emit_xgather(q) for q in range(2, min(DIST, R))}
            htt = {0: emit_fc1(xq01, base=0), 1: emit_fc1(xq01, base=16)}
            xq = {}
            for r in range(R):
                if r + 2 < R:
                    xq[r + 2] = emit_xcast(xb.pop(r + 2))
                hT = emit_hT(htt.pop(r))
                if r + 2 < R:
                    htt[r + 2] = emit_fc1(xq.pop(r + 2))
                emit_fc2(hT, r)
                if r + DIST < R:
                    xb[r + DIST] = emit_xgather(r + DIST)

    nc.compile()
    _PROG[key] = nc
    return nc


def _node_weight_like_reference(rep, n_per_bag):
    """Bit-faithful mirror of the reference's eager node_weight computation
    (reference runs on CPU jax; mirror that exactly)."""
    import jax
    import jax.numpy as jnp
    cpu = jax.local_devices(backend="cpu")[0]
    with jax.default_device(cpu):
        d = rep.shape[-1]
        bags = jnp.asarray(np.ascontiguousarray(rep, dtype=np.float32)).reshape(-1, n_per_bag, d)
        norms = jnp.linalg.norm(bags, axis=-1)
        gram = jnp.einsum('bnd,bmd->bnm', bags, bags)
        sims = gram / jnp.maximum(norms[:, :, None] * norms[:, None, :], 1e-8)
        node_distance = sims.sum(axis=1)
        node_weight = jax.nn.softmax(node_distance, axis=-1)
        return np.asarray(node_weight).astype(np.float32)


def _huffman_schedule(w):
    """Replay the reference scan's weight bookkeeping (exact f32) and emit
    per-bag merge operand nodes: leaves 0..63, merge t -> 64+t."""
    B, n = w.shape
    wref = w.copy()
    alive = np.ones((B, n), bool)
    prov = np.tile(np.arange(n, dtype=np.int64), (B, 1))
    ar = np.arange(B)
    gl = np.zeros((B, n - 1), np.int64)
    gr = np.zeros((B, n - 1), np.int64)
    INF = np.float32(np.inf)
    for t in range(n - 1):
        wm = np.where(alive, wref, INF)
        i1 = np.argmin(wm, axis=1)
        wm2 = wm.copy()
        wm2[ar, i1] = INF
        i2 = np.argmin(wm2, axis=1)
        gl[:, t] = prov[ar, i1]
        gr[:, t] = prov[ar, i2]
        wref[ar, i1] = wm[ar, i1] + wm[ar, i2]
        alive[ar, i2] = False
        prov[ar, i1] = n + t
    return gl, gr


def _pack_rounds(gl, gr, n=NN, dist=DIST):
    """List-schedule each bag's n-1 merges into pair-rounds (2 independent
    merges per round; children must be done <= r-dist; priority = longest
    path to root).  The root merge is pinned last.  Returns
    (rounds_of, slot_of, R_full)."""
    B, m = gl.shape
    rounds_of = np.zeros((B, m), np.int64)
    slot_of = np.zeros((B, m), np.int64)
    last_nonroot = 0
    root_child_max = 0
    for b in range(B):
        cl, cr = gl[b], gr[b]
        parents = np.full(m, -1, np.int64)
        ndep = np.zeros(m, np.int32)
        for j in range(m):
            for s in (cl[j], cr[j]):
                if s >= n:
                    ndep[j] += 1
                    parents[s - n] = j
        height = np.zeros(m, np.int64)
        for j in range(m - 1, -1, -1):
            p = parents[j]
            if p >= 0:
                height[j] = height[p] + 1
        done = np.full(m, 10**9, np.int64)
        remaining = ndep.copy()
        scheduled = 0
        r = 0
        while scheduled < m:
            ready = [j for j in range(m)
                     if remaining[j] == 0 and done[j] == 10**9
                     and all((s < n or done[s - n] <= r - dist) for s in (cl[j], cr[j]))]
            ready.sort(key=lambda j: (-height[j], j))
            for s_idx, j in enumerate(ready[:2]):
                rounds_of[b, j] = r
                slot_of[b, j] = s_idx
                done[j] = r
                scheduled += 1
                p = parents[j]
                if p >= 0:
                    remaining[p] -= 1
            r += 1
            assert r < 8 * m, "packer stuck"
        last_nonroot = max(last_nonroot, rounds_of[b, :m - 1].max())
        for s in (cl[m - 1], cr[m - 1]):
            if s >= n:
                root_child_max = max(root_child_max, int(rounds_of[b, s - n]))
    root_round = max(last_nonroot + 1, root_child_max + dist)
    rounds_of[:, m - 1] = root_round
    slot_of[:, m - 1] = 0
    R = root_round + 1
    for b in range(B):
        for j in range(m):
            r = rounds_of[b, j]
            for s in (gl[b, j], gr[b, j]):
                if s >= n:
                    assert rounds_of[b, s - n] <= r - dist, \
                        f"dist-{dist} violated: bag {b} merge {j}"
    return rounds_of, slot_of, R


def _wrap16(arr):
    """Pack a flat int array of gather positions into the SWDGE wrapped-16
    idx layout [128, n/16]: position n -> [n%16, n//16], replicated across
    the 8 partition groups."""
    ncols = len(arr) // 16
    block = arr.astype(np.int16).reshape(ncols, 16).T  # [16, ncols]
    return np.tile(block, (8, 1))  # [128, ncols]


def _prepare(rep, fc1_w, fc1_b, fc2_w, fc2_b, rel_emb, n_per_bag, **kw):
    n_per_bag = int(n_per_bag)
    assert n_per_bag == NN and rep.shape[-1] == D
    rep = np.ascontiguousarray(rep, dtype=np.float32)

    w = _node_weight_like_reference(rep, n_per_bag)
    gl, gr = _huffman_schedule(w)
    rounds_of, slot_of, R_full = _pack_rounds(gl, gr)
    B, m = gl.shape

    # device cut: keep only the rounds where EVERY bag has 2 merges
    percnt = np.zeros((B, R_full), np.int64)
    for b in range(B):
        for j in range(m):
            percnt[b, rounds_of[b, j]] += 1
    fullr = (percnt == 2).all(axis=0)
    R = int(np.argmin(fullr)) if not fullr.all() else R_full
    assert R > DIST

    SL = 64 + 2 * (R - DIST)
    zb = (not np.any(np.asarray(fc1_b))) and (not np.any(np.asarray(fc2_b)))
    merge_slot = 64 + 2 * rounds_of + slot_of          # (B, m); valid r < R-DIST

    # host tail: merges at rounds >= R.  Their device-side children must be
    # exactly the outputs of rounds R-DIST..R-1 (those are never consumed on
    # device and are DMA'd to the output tensor).
    host_merges = [[j for j in range(m) if rounds_of[b, j] >= R] for b in range(B)]
    for b in range(B):
        hs = set(host_merges[b])
        for j in host_merges[b]:
            for s in (gl[b, j], gr[b, j]):
                if s >= NN and (s - NN) not in hs:
                    assert rounds_of[b, s - NN] >= R - DIST, \
                        f"host child of bag {b} produced too early"
                else:
                    assert s >= NN, f"leaf child in host tail of bag {b}"

    nc = _build_program(R, SL, zb)

    import ml_dtypes
    f8 = ml_dtypes.float8_e4m3fn
    w1t = np.ascontiguousarray(np.asarray(fc1_w, np.float32).T).astype(f8)   # (2D, D)
    w2t = np.ascontiguousarray(np.asarray(fc2_w, np.float32).T).astype(f8)   # (D, D)
    b1b = np.ascontiguousarray(np.broadcast_to(np.asarray(fc1_b, np.float32), (128, D)))
    b2b = np.ascontiguousarray(np.broadcast_to(np.asarray(fc2_b, np.float32), (128, D)))
    ident = np.eye(128, dtype=ml_dtypes.bfloat16)

    def node_row(b, node):
        lb = b % NB
        return lb * SL + (node if node < NN else merge_slot[b, node - NN])

    in_maps = []
    for c in range(NCORES):
        b0 = c * NB
        gidx = np.zeros((128, 16 * R), np.int16)
        for q in range(R):
            for h in (0, 1):
                child = gl if h == 0 else gr
                arr = np.zeros(128, np.int64)
                for lb in range(NB):
                    b = b0 + lb
                    js = np.where(rounds_of[b] == q)[0]
                    assert len(js) == 2
                    for j in js:
                        s = slot_of[b, j]
                        arr[s * NB + lb] = node_row(b, int(child[b, j]))
                gidx[:, 16 * q + 8 * h:16 * q + 8 * h + 8] = _wrap16(arr)

        feats = np.zeros((NB * SL, D), ml_dtypes.bfloat16)
        leaves = np.tanh(rep[b0 * NN:(b0 + NB) * NN].reshape(NB, NN, D)).astype(ml_dtypes.bfloat16)
        feats.reshape(NB, SL, D)[:, :NN, :] = leaves

        # host-pre-gathered/transposed/cast operands for rounds 0/1 (all
        # children are leaves when DIST >= 2): xq01[p, 16q+8h+c, j] =
        # feats[child_row(q,h,j), 128c+p], bf16 -> fp8 like the device cast
        x01 = np.zeros((128, 32, 128), ml_dtypes.float8_e4m3fn)
        for q in (0, 1):
            for h in (0, 1):
                child = gl if h == 0 else gr
                arr = np.zeros(128, np.int64)
                for lb in range(NB):
                    b = b0 + lb
                    js = np.where(rounds_of[b] == q)[0]
                    for j in js:
                        ch = int(child[b, j])
                        assert ch < NN, "round 0/1 child must be a leaf"
                        arr[slot_of[b, j] * NB + lb] = lb * SL + ch
                rows = feats[arr].astype(np.float32)            # (128, 1024)
                xt = rows.T.reshape(8, 128, 128)                # (c, p, j)
                for c in range(8):
                    x01[:, 16 * q + 8 * h + c, :] = xt[c].astype(ml_dtypes.bfloat16).astype(ml_dtypes.float8_e4m3fn)

        in_maps.append({
            "feats": feats,
            "w1t": w1t, "w2t": w2t, "xq01": np.ascontiguousarray(x01.reshape(128, 32 * 128)),
            "b1b": b1b, "b2b": b2b, "gidx": gidx, "ident": ident,
        })

    tail = {
        "gl": gl, "gr": gr, "rounds_of": rounds_of, "slot_of": slot_of,
        "R": R, "host_merges": host_merges,
    }
    return nc, in_maps, tail


def _host_tail(res, tail, rep, fc1_w, fc1_b, fc2_w, fc2_b, rel_emb):
    """Replay the chain-bound tail merges in f32 and produce the output."""
    gl, gr = tail["gl"], tail["gr"]
    rounds_of, slot_of = tail["rounds_of"], tail["slot_of"]
    host_merges = tail["host_merges"]
    R = tail["R"]
    B, m = gl.shape
    w1 = np.asarray(fc1_w, np.float32)    # (D, 2D)
    w2 = np.asarray(fc2_w, np.float32)    # (D, D)
    b1 = np.asarray(fc1_b, np.float32)
    b2 = np.asarray(fc2_b, np.float32)
    rel = np.asarray(rel_emb, np.float32)

    # tanh'd features of the last-DIST-round device merges, per (bag, node)
    feat = {}
    for c in range(NCORES):
        fout = np.asarray(res.results[c]["out"]).astype(np.float32)  # (NB, 2*DIST, D)
        for lb in range(NB):
            b = c * NB + lb
            for j in range(m):
                r = rounds_of[b, j]
                if R - DIST <= r < R:
                    t = 2 * (r - (R - DIST)) + slot_of[b, j]
                    feat[(b, NN + j)] = fout[lb, t]

    groups = {}
    for b in range(B):
        for j in host_merges[b]:
            groups.setdefault(int(rounds_of[b, j]), []).append((b, j))
    root_feat = np.zeros((B, D), np.float32)
    for q in sorted(groups):
        items = groups[q]
        x = np.empty((len(items), 2 * D), np.float32)
        for i, (b, j) in enumerate(items):
            x[i, :D] = feat[(b, int(gl[b, j]))]
            x[i, D:] = feat[(b, int(gr[b, j]))]
        h = np.tanh(x @ w1.T + b1)
        f = h @ w2.T + b2                 # raw features of the new nodes
        for i, (b, j) in enumerate(items):
            if j == m - 1:
                root_feat[b] = f[i]
            else:
                feat[(b, NN + j)] = np.tanh(f[i])
    scores = root_feat @ rel.T
    out = 1.0 / (1.0 + np.exp(-scores, dtype=np.float64))
    return np.ascontiguousarray(out.astype(np.float32))


def kernel(rep, fc1_w, fc1_b, fc2_w, fc2_b, rel_emb, n_per_bag, **kw):
    nc, in_maps, tail = _prepare(rep, fc1_w, fc1_b, fc2_w, fc2_b, rel_emb, n_per_bag)
    from concourse import bass_utils
    res = bass_utils.run_bass_kernel_spmd(nc, in_maps, core_ids=list(range(NCORES)))
    return _host_tail(res, tail, rep, fc1_w, fc1_b, fc2_w, fc2_b, rel_emb)
